# revision 1
# baseline (speedup 1.0000x reference)
"""HeadConvAttention Trainium2 Bass kernel.

Reference computation (per batch b):
    scores[h,q,k] = <xq[h,q,:], xk[h,k,:]> / sqrt(D)
    mixed[g,q,k]  = sum_h W[g,h] * scores[h,q,k]
    probs         = softmax(mixed + causal_mask, axis=k)
    out[q,g,d]    = sum_k probs[g,q,k] * xv[g,k,d]

Sharding: 8 cores = batch(4) x q-parity(2). Each core handles q rows
{parity, parity+2, ...} (512 rows) of one batch element — strided q keeps
the causal workload balanced across parities.

On-chip pipeline per core (all matmuls on PE):
  - transpose Q,K tiles to [d, s] layout (2 heads packed per 128 partitions)
  - QK^T in float32r, two row-tiled (K=64) matmuls per head pair
  - "fold" transpose: scores [q, (h,kc8)] -> [(h,kc8), q] per 8-k-block
  - mixing matmul: lhsT=folded, rhs=block-diag W (bf16) -> mixed [q, (g,kc8)]
  - causal mask add (precomputed on host), exp on ScalarE (no max subtraction:
    mixed ~ N(0, 0.34), overflow impossible in f32), probs stored bf16
  - PV: per (g, 128-k-block) transpose probs -> [k, q], matmul with V [k, d]
  - divide by row-sum at the very end on the [q, d] tile
"""

import numpy as np

B, H, S, D = 4, 16, 1024, 64
QC = S // 2          # q rows per core
NQT = QC // 128      # q tiles per core (4)
NEG = -1.0e30

_compiled = {}
TRACE = False          # set True to capture an NTFF profile on the next call
LAST_EXEC_NS = None
LAST_PROFILE = None


def _build_nc(causal: int):
    import contextlib

    import concourse.bacc as bacc
    import concourse.bass as bass
    import concourse.mybir as mybir
    import concourse.tile as tile

    dt = mybir.dt
    f32, f32r, bf16 = dt.float32, dt.float32r, dt.bfloat16
    AF = mybir.ActivationFunctionType
    AX = mybir.AxisListType

    nc = bacc.Bacc("TRN2", target_bir_lowering=False, debug=False, num_devices=8)

    xq_c = nc.dram_tensor("xq_c", [H, QC, D], f32, kind="ExternalInput")
    xk_c = nc.dram_tensor("xk_c", [H, S, D], f32, kind="ExternalInput")
    xv_c = nc.dram_tensor("xv_c", [H, S, D], f32, kind="ExternalInput")
    wblk = nc.dram_tensor("wblk", [128, 128], f32, kind="ExternalInput")
    cmask = nc.dram_tensor("cmask", [128, 256], f32, kind="ExternalInput")
    ident = nc.dram_tensor("ident", [128, 128], f32, kind="ExternalInput")
    out_c = nc.dram_tensor("out_c", [QC, H, D], f32, kind="ExternalOutput")

    # per-q-tile causal extents (in k units of 8 and 128)
    if causal:
        kmax = [256 * (j + 1) for j in range(NQT)]
    else:
        kmax = [S for _ in range(NQT)]
    nkb = [km // 8 for km in kmax]      # 8-k-blocks per q tile
    nkblk = [km // 128 for km in kmax]  # 128-k-blocks per q tile
    nkt = [(km + 511) // 512 for km in kmax]  # 512-k-tiles per q tile
    max_probs_cols = max(nkb) * 128

    with tile.TileContext(nc) as tc:
        with contextlib.ExitStack() as ctx:
            const = ctx.enter_context(tc.tile_pool(name="const", bufs=1))
            stage = ctx.enter_context(tc.tile_pool(name="stage", bufs=3))
            persist = ctx.enter_context(tc.tile_pool(name="persist", bufs=1))
            sc_pool = ctx.enter_context(tc.tile_pool(name="scores", bufs=2))
            sm_pool = ctx.enter_context(tc.tile_pool(name="small", bufs=4))
            # PSUM budget: 8 banks total. qk 3 + t 2 + mix 2 + out 1 = 8.
            ps_qk = ctx.enter_context(tc.tile_pool(name="ps_qk", bufs=3, space="PSUM"))
            ps_t = ctx.enter_context(tc.tile_pool(name="ps_t", bufs=2, space="PSUM"))
            ps_mix = ctx.enter_context(tc.tile_pool(name="ps_mix", bufs=2, space="PSUM"))
            ps_out = ctx.enter_context(tc.tile_pool(name="ps_out", bufs=1, space="PSUM"))

            # ---- constants ----
            id_f32 = const.tile([128, 128], f32, tag="id_f32")
            nc.sync.dma_start(out=id_f32, in_=ident[:, :])
            id_bf = const.tile([128, 128], bf16, tag="id_bf")
            nc.vector.tensor_copy(id_bf, id_f32)
            wblk_f = const.tile([128, 128], f32, tag="wblk_f")
            nc.sync.dma_start(out=wblk_f, in_=wblk[:, :])
            wblk_bf = const.tile([128, 128], bf16, tag="wblk_bf")
            nc.vector.tensor_copy(wblk_bf, wblk_f)
            cmask_sb = const.tile([128, 256], f32, tag="cmask")
            nc.sync.dma_start(out=cmask_sb, in_=cmask[:, :])

            # ---- Q/K transposes to [d, s] (2 heads per 128 partitions) ----
            # QT2[pair]: [(hl*64+d), q=512], KT2[pair]: [(hl*64+d), k=1024]
            qt2 = [persist.tile([128, QC], f32r, tag=f"qt2_{p}", name=f"qt2_{p}") for p in range(8)]
            kt2 = [persist.tile([128, S], f32r, tag=f"kt2_{p}", name=f"kt2_{p}") for p in range(8)]
            for p in range(8):
                for t in range(NQT):
                    qx = stage.tile([128, 2, 64], f32, tag="qx")
                    nc.sync.dma_start(
                        out=qx,
                        in_=xq_c[2 * p : 2 * p + 2, 128 * t : 128 * (t + 1), :].rearrange(
                            "h q d -> q h d"
                        ),
                    )
                    pt = ps_t.tile([128, 128], f32, tag="t", name="pt")
                    nc.tensor.transpose(pt, qx.rearrange("q h d -> q (h d)"), id_f32)
                    nc.scalar.copy(qt2[p][:, 128 * t : 128 * (t + 1)], pt)
                for t in range(S // 128):
                    kx = stage.tile([128, 2, 64], f32, tag="kx")
                    nc.sync.dma_start(
                        out=kx,
                        in_=xk_c[2 * p : 2 * p + 2, 128 * t : 128 * (t + 1), :].rearrange(
                            "h k d -> k h d"
                        ),
                    )
                    pt = ps_t.tile([128, 128], f32, tag="t", name="pt")
                    nc.tensor.transpose(pt, kx.rearrange("k h d -> k (h d)"), id_f32)
                    nc.scalar.copy(kt2[p][:, 128 * t : 128 * (t + 1)], pt)

            # ---- V: [k=128, (g, kblk, d)] bf16 ----
            v_bf = persist.tile([128, H, 8, 64], bf16, tag="v_bf")
            for g in range(H):
                vx = stage.tile([128, 8, 64], f32, tag="vx")
                nc.sync.dma_start(
                    out=vx, in_=xv_c[g, :, :].rearrange("(kb kp) d -> kp kb d", kp=128)
                )
                nc.vector.tensor_copy(v_bf[:, g, :, :], vx)

            # ---- main loop over q tiles ----
            for j in range(NQT):
                # scores_store: [q=128, (kb_local, h, kc)] f32 per 512-k-tile
                probs = persist.tile([128, max_probs_cols], bf16, tag="probs")
                lacc = sm_pool.tile([128, H], f32, tag="lacc")

                for kt in range(nkt[j]):
                    klen = min(512, kmax[j] - 512 * kt)
                    nkb_t = klen // 8
                    sc = sc_pool.tile([128, 64, H, 8], f32, tag="sc")
                    for p in range(8):
                        pq0 = ps_qk.tile([128, 512], f32, tag="pqk", name="pq0")[:, 0:klen]
                        pq1 = ps_qk.tile([128, 512], f32, tag="pqk", name="pq1")[:, 0:klen]
                        nc.tensor.matmul(
                            pq0,
                            qt2[p][0:64, 128 * j : 128 * (j + 1)],
                            kt2[p][0:64, 512 * kt : 512 * kt + klen],
                            start=True,
                            stop=True,
                            tile_position=(0, 0),
                        )
                        nc.tensor.matmul(
                            pq1,
                            qt2[p][64:128, 128 * j : 128 * (j + 1)],
                            kt2[p][64:128, 512 * kt : 512 * kt + klen],
                            start=True,
                            stop=True,
                            tile_position=(64, 0),
                        )
                        # scatter into [kb, h, kc] layout
                        nc.scalar.copy(
                            sc[:, 0:nkb_t, 2 * p, :],
                            pq0.rearrange("q (kb kc) -> q kb kc", kc=8),
                        )
                        nc.scalar.copy(
                            sc[:, 0:nkb_t, 2 * p + 1, :],
                            pq1.rearrange("q (kb kc) -> q kb kc", kc=8),
                        )

                    for kbl in range(nkb_t):
                        kb = 64 * kt + kbl
                        pf = ps_t.tile([128, 128], f32, tag="t", name="pf")
                        nc.tensor.transpose(
                            pf,
                            sc[:, kbl, :, :].rearrange("q h kc -> q (h kc)"),
                            id_f32,
                        )
                        fold_bf = sm_pool.tile([128, 128], bf16, tag="fold_bf")
                        nc.vector.tensor_copy(fold_bf, pf)
                        pm = ps_mix.tile([128, 128], f32, tag="mix")
                        nc.tensor.matmul(pm, fold_bf, wblk_bf, start=True, stop=True)
                        if causal and kb >= 32 * j:
                            t_loc = kb - 32 * j
                            mrow = cmask_sb[:, 8 * t_loc : 8 * t_loc + 8]
                            mask_b = bass.AP(
                                tensor=mrow.tensor,
                                offset=mrow.offset,
                                ap=[mrow.ap[0], [0, H], mrow.ap[1]],
                            )
                            nc.vector.tensor_add(
                                pm.rearrange("q (g kc) -> q g kc", kc=8),
                                pm.rearrange("q (g kc) -> q g kc", kc=8),
                                mask_b,
                            )
                        # exp -> probs[:, (g, kb, kc)] bf16
                        nc.scalar.activation(
                            probs.rearrange("q (g kb kc) -> q g kb kc", g=H, kc=8)[
                                :, :, kb, :
                            ],
                            pm.rearrange("q (g kc) -> q g kc", kc=8),
                            AF.Exp,
                        )

                # row sums per g: reduce over (kb, kc)
                nc.vector.reduce_sum(
                    lacc,
                    probs.rearrange("q (g k) -> q g k", g=H)[:, :, 0 : 8 * nkb[j]],
                    axis=AX.X,
                )
                linv = sm_pool.tile([128, H], f32, tag="linv")
                nc.vector.reciprocal(linv, lacc)

                out_sb = sc_pool.tile([128, H, 64], f32, tag="out_sb")
                for g in range(H):
                    po = ps_out.tile([128, 64], f32, tag="pv_out")
                    for kblk in range(nkblk[j]):
                        pp = ps_t.tile([128, 128], bf16, tag="t", name="pp")
                        nc.tensor.transpose(
                            pp,
                            probs.rearrange("q (g k) -> q g k", g=H)[
                                :, g, 128 * kblk : 128 * (kblk + 1)
                            ],
                            id_bf,
                        )
                        pvt_bf = sm_pool.tile([128, 128], bf16, tag="pvt_bf")
                        nc.vector.tensor_copy(pvt_bf, pp)  # bf16 PSUM -> bf16 SBUF
                        nc.tensor.matmul(
                            po,
                            pvt_bf,
                            v_bf[:, g, kblk, :],
                            start=(kblk == 0),
                            stop=(kblk == nkblk[j] - 1),
                        )
                    nc.vector.tensor_scalar_mul(
                        out_sb[:, g, :], po, linv[:, g : g + 1]
                    )
                nc.sync.dma_start(
                    out=out_c[128 * j : 128 * (j + 1), :, :], in_=out_sb
                )

    nc.compile()
    return nc


def _get_nc(causal: int):
    key = int(causal)
    if key not in _compiled:
        _compiled[key] = _build_nc(key)
    return _compiled[key]


def kernel(xq, xk, xv, W, causal):
    from concourse.bass_utils import run_bass_kernel_spmd

    causal = int(np.asarray(causal))
    nc = _get_nc(causal)

    W = np.asarray(W, dtype=np.float32)
    # block-diagonal mixing weight: wblk[8h+kc, 8g+kc] = W[g,h] / 8
    wblk = np.zeros((128, 128), dtype=np.float32)
    for kc in range(8):
        wblk[kc::8, kc::8] = W.T / 8.0
    ident = np.eye(128, dtype=np.float32)

    in_maps = []
    for cid in range(8):
        b, par = divmod(cid, 2)
        # cmask[qc', 8t+kc] = 0 if 8t+kc <= 2qc'+par else NEG
        qcp = np.arange(128)[:, None]
        kk = np.arange(256)[None, :]
        cm = np.where(kk <= 2 * qcp + par, 0.0, NEG).astype(np.float32)
        in_maps.append(
            {
                "xq_c": np.ascontiguousarray(xq[b, :, par::2, :], dtype=np.float32),
                "xk_c": np.ascontiguousarray(xk[b], dtype=np.float32),
                "xv_c": np.ascontiguousarray(xv[b], dtype=np.float32),
                "wblk": wblk,
                "cmask": cm,
                "ident": ident,
            }
        )

    global LAST_EXEC_NS, LAST_PROFILE
    res = run_bass_kernel_spmd(nc, in_maps, list(range(8)), trace=TRACE)
    if res.exec_time_ns is not None:
        LAST_EXEC_NS = res.exec_time_ns
        LAST_PROFILE = res.profile_json
    out = np.empty((B, S, H, D), dtype=np.float32)
    for cid in range(8):
        b, par = divmod(cid, 2)
        out[b, par::2, :, :] = res.results[cid]["out_c"]
    return out



# revision 7
# speedup vs baseline: 1.4492x; 1.4492x over previous
"""HeadConvAttention Trainium2 Bass kernel.

Reference computation (per batch b):
    scores[h,q,k] = <xq[h,q,:], xk[h,k,:]> / sqrt(D)
    mixed[g,q,k]  = sum_h W[g,h] * scores[h,q,k]
    probs         = softmax(mixed + causal_mask, axis=k)
    out[q,g,d]    = sum_k probs[g,q,k] * xv[g,k,d]

Sharding: 8 cores = batch(4) x q-parity(2). Each core handles q rows
{parity, parity+2, ...} (512 rows) of one batch element — strided q keeps
the causal workload balanced across parities.

On-chip pipeline per core (all matmuls on PE):
  - transpose Q,K tiles to [d, s] layout (2 heads packed per 128 partitions)
  - QK^T in float32r (1 cyc/row at >=256 cols), two matmuls per head pair
  - scatter QK PSUM -> sc [q, h, kb, kc] bf16 (cast on Act engine)
  - "fold" transpose per 8-k-block: [q, (h,kc8)] -> [(h,kc8), q] in bf16,
    8 transposes per PSUM bank, one batched DVE copy to SBUF
  - mixing matmul: lhsT=fold, rhs=block-diag W (bf16) -> pm [q, (kb4, g, kc)]
  - exp on ScalarE over whole [128, 512] PSUM banks -> probs [q, kb, g, kc]
    bf16 (no max subtraction: mixed ~ N(0, 0.34), overflow impossible)
  - causal masking: post-exp multiplicative 0/1 mask on GpSimd (diag blocks)
  - PV: per g, transpose probs -> [k, q] (batched in PSUM), matmul with
    V augmented by a ones column -> [q, 64 out + 1 row-sum]
  - divide by the row-sum column at the very end
"""

import numpy as np

B, H, S, D = 4, 16, 1024, 64
QC = S // 2          # q rows per core
NQT = QC // 128      # q tiles per core (4)

_compiled = {}
TRACE = False          # set True to capture an NTFF profile on the next call
LAST_EXEC_NS = None
LAST_PROFILE = None


def _build_nc(causal: int):
    import contextlib

    import concourse.bacc as bacc
    import concourse.bass as bass
    import concourse.mybir as mybir
    import concourse.tile as tile

    dt = mybir.dt
    f32, f32r, bf16 = dt.float32, dt.float32r, dt.bfloat16
    AF = mybir.ActivationFunctionType

    nc = bacc.Bacc("TRN2", target_bir_lowering=False, debug=False, num_devices=8)

    xq_c = nc.dram_tensor("xq_c", [H, QC, D], f32, kind="ExternalInput")
    xk_c = nc.dram_tensor("xk_c", [H, S, D], f32, kind="ExternalInput")
    xv_c = nc.dram_tensor("xv_c", [H, S, D], f32, kind="ExternalInput")
    wblk = nc.dram_tensor("wblk", [128, 128], f32, kind="ExternalInput")
    cmask = nc.dram_tensor("cmask", [128, 256], f32, kind="ExternalInput")
    ident = nc.dram_tensor("ident", [128, 128], f32, kind="ExternalInput")
    out_c = nc.dram_tensor("out_c", [QC, H, D], f32, kind="ExternalOutput")

    # per-q-tile causal extents (in k units of 8 and 128)
    if causal:
        kmax = [256 * (j + 1) for j in range(NQT)]
    else:
        kmax = [S for _ in range(NQT)]
    nkb = [km // 8 for km in kmax]      # 8-k-blocks per q tile
    nkblk = [km // 128 for km in kmax]  # 128-k-blocks per q tile
    nkt = [(km + 511) // 512 for km in kmax]  # 512-k-tiles per q tile
    max_nkb = max(nkb)

    with tile.TileContext(nc) as tc:
        with contextlib.ExitStack() as ctx:
            const = ctx.enter_context(tc.tile_pool(name="const", bufs=1))
            stage = ctx.enter_context(tc.tile_pool(name="stage", bufs=3))
            persist = ctx.enter_context(tc.tile_pool(name="persist", bufs=1))
            sc_pool = ctx.enter_context(tc.tile_pool(name="scores", bufs=2))
            fold_pool = ctx.enter_context(tc.tile_pool(name="fold", bufs=3))
            pvt_pool = ctx.enter_context(tc.tile_pool(name="pvt", bufs=3))
            out_pool = ctx.enter_context(tc.tile_pool(name="outp", bufs=2))
            sm_pool = ctx.enter_context(tc.tile_pool(name="small", bufs=4))
            # PSUM budget: 8 banks. qk 2 + ft 2 + mix 2 + out 2 = 8.
            ps_qk = ctx.enter_context(tc.tile_pool(name="ps_qk", bufs=2, space="PSUM"))
            ps_ft = ctx.enter_context(tc.tile_pool(name="ps_ft", bufs=2, space="PSUM"))
            ps_mix = ctx.enter_context(tc.tile_pool(name="ps_mix", bufs=2, space="PSUM"))
            ps_out = ctx.enter_context(tc.tile_pool(name="ps_out", bufs=2, space="PSUM"))

            # ---- constants ----
            id_f32 = const.tile([128, 128], f32, tag="id_f32")
            nc.sync.dma_start(out=id_f32, in_=ident[:, :])
            id_bf = const.tile([128, 128], bf16, tag="id_bf")
            nc.vector.tensor_copy(id_bf, id_f32)
            wblk_f = const.tile([128, 128], f32, tag="wblk_f")
            nc.sync.dma_start(out=wblk_f, in_=wblk[:, :])
            wblk_bf = const.tile([128, 128], bf16, tag="wblk_bf")
            nc.vector.tensor_copy(wblk_bf, wblk_f)
            cmask_f = const.tile([128, 256], f32, tag="cmask_f")
            nc.sync.dma_start(out=cmask_f, in_=cmask[:, :])
            cm01 = const.tile([128, 256], bf16, tag="cm01")
            nc.vector.tensor_copy(cm01, cmask_f)

            # ---- Q/K transposes to [d, s] (2 heads per 128 partitions) ----
            # QT2[pair]: [(hl*64+d), q=512], KT2[pair]: [(hl*64+d), k=1024]
            qt2 = [persist.tile([128, QC], f32r, tag=f"qt2_{p}", name=f"qt2_{p}") for p in range(8)]
            kt2 = [persist.tile([128, S], f32r, tag=f"kt2_{p}", name=f"kt2_{p}") for p in range(8)]
            for p in range(8):
                for t in range(NQT):
                    qx = stage.tile([128, 2, 64], f32, tag="qx")
                    nc.sync.dma_start(
                        out=qx,
                        in_=xq_c[2 * p : 2 * p + 2, 128 * t : 128 * (t + 1), :].rearrange(
                            "h q d -> q h d"
                        ),
                    )
                    pt = ps_qk.tile([128, 512], f32, tag="pqk", name="ptq")[:, 0:128]
                    nc.tensor.transpose(pt, qx.rearrange("q h d -> q (h d)"), id_f32)
                    nc.scalar.copy(qt2[p][:, 128 * t : 128 * (t + 1)], pt)
                for t in range(S // 128):
                    kx = stage.tile([128, 2, 64], f32, tag="kx")
                    nc.sync.dma_start(
                        out=kx,
                        in_=xk_c[2 * p : 2 * p + 2, 128 * t : 128 * (t + 1), :].rearrange(
                            "h k d -> k h d"
                        ),
                    )
                    pt = ps_qk.tile([128, 512], f32, tag="pqk", name="ptk")[:, 0:128]
                    nc.tensor.transpose(pt, kx.rearrange("k h d -> k (h d)"), id_f32)
                    nc.scalar.copy(kt2[p][:, 128 * t : 128 * (t + 1)], pt)

            # ---- V: [k=128, (g, kblk, d+1)] bf16, last column = 1.0 ----
            v_bf = persist.tile([128, H, 8, 65], bf16, tag="v_bf")
            nc.vector.memset(v_bf[:, :, :, 64:65], 1.0)
            for g in range(H):
                vx = stage.tile([128, 8, 64], f32, tag="vx")
                nc.sync.dma_start(
                    out=vx, in_=xv_c[g, :, :].rearrange("(kb kp) d -> kp kb d", kp=128)
                )
                nc.vector.tensor_copy(v_bf[:, g, :, 0:64], vx)

            probs = persist.tile([128, H, max_nkb, 8], bf16, tag="probs")

            # ---- main loop over q tiles ----
            for j in range(NQT):
                # phase A: scores -> fold -> mix -> exp -> probs
                for kt in range(nkt[j]):
                    klen = min(512, kmax[j] - 512 * kt)
                    nkb_t = klen // 8
                    sc = sc_pool.tile([128, 64, H, 8], bf16, tag="sc")
                    for p in range(8):
                        pq0 = ps_qk.tile([128, 512], f32, tag="pqk", name="pq0")[:, 0:klen]
                        pq1 = ps_qk.tile([128, 512], f32, tag="pqk", name="pq1")[:, 0:klen]
                        nc.tensor.matmul(
                            pq0,
                            qt2[p][0:64, 128 * j : 128 * (j + 1)],
                            kt2[p][0:64, 512 * kt : 512 * kt + klen],
                            start=True,
                            stop=True,
                            tile_position=(0, 0),
                        )
                        nc.tensor.matmul(
                            pq1,
                            qt2[p][64:128, 128 * j : 128 * (j + 1)],
                            kt2[p][64:128, 512 * kt : 512 * kt + klen],
                            start=True,
                            stop=True,
                            tile_position=(64, 0),
                        )
                        nc.scalar.copy(
                            sc[:, 0:nkb_t, 2 * p, :],
                            pq0.rearrange("q (kb kc) -> q kb kc", kc=8),
                        )
                        nc.scalar.copy(
                            sc[:, 0:nkb_t, 2 * p + 1, :],
                            pq1.rearrange("q (kb kc) -> q kb kc", kc=8),
                        )

                    for grp in range(nkb_t // 8):
                        ft = ps_ft.tile([128, 8, 128], bf16, tag="ft", name="ft")
                        for i in range(8):
                            kbl = grp * 8 + i
                            nc.tensor.transpose(
                                ft[:, i, :],
                                sc[:, kbl, :, :].rearrange("q h kc -> q (h kc)"),
                                id_bf,
                            )
                        fold_sb = fold_pool.tile([128, 8, 128], bf16, tag="fold_sb")
                        nc.vector.tensor_copy(fold_sb, ft)
                        for half in range(2):
                            kb0 = 64 * kt + grp * 8 + half * 4
                            pm = ps_mix.tile([128, 4, H, 8], f32, tag="mix")
                            for i2 in range(4):
                                nc.tensor.matmul(
                                    pm[:, i2, :, :].rearrange("q g kc -> q (g kc)"),
                                    fold_sb[:, half * 4 + i2, :],
                                    wblk_bf,
                                    start=True,
                                    stop=True,
                                )
                            nc.scalar.activation(
                                probs[:, :, kb0 : kb0 + 4, :].rearrange(
                                    "q g kb kc -> q kb g kc"
                                ),
                                pm,
                                AF.Exp,
                            )
                            if causal:
                                for i2 in range(4):
                                    kb = kb0 + i2
                                    if kb >= 32 * j:
                                        t_loc = kb - 32 * j
                                        mrow = cm01[:, 8 * t_loc : 8 * t_loc + 8]
                                        mask_b = bass.AP(
                                            tensor=mrow.tensor,
                                            offset=mrow.offset,
                                            ap=[mrow.ap[0], [0, H], mrow.ap[1]],
                                        )
                                        nc.gpsimd.tensor_mul(
                                            probs[:, :, kb, :],
                                            probs[:, :, kb, :],
                                            mask_b,
                                        )

                # phase B: probs^T -> PV (with ones column row-sums) -> scale
                out_sb = out_pool.tile([128, H, 64], f32, tag="out_sb")
                for g in range(H):
                    pt = ps_ft.tile([128, 8, 128], bf16, tag="ft", name="pt")
                    for kblk in range(nkblk[j]):
                        nc.tensor.transpose(
                            pt[:, kblk, :],
                            probs[:, g, 16 * kblk : 16 * (kblk + 1), :].rearrange(
                                "q kb kc -> q (kb kc)"
                            ),
                            id_bf,
                        )
                    pvt = pvt_pool.tile([128, 8, 128], bf16, tag="pvt")
                    nc.vector.tensor_copy(
                        pvt[:, 0 : nkblk[j], :], pt[:, 0 : nkblk[j], :]
                    )
                    po = ps_out.tile([128, 65], f32, tag="pv_out")
                    for kblk in range(nkblk[j]):
                        nc.tensor.matmul(
                            po,
                            pvt[:, kblk, :],
                            v_bf[:, g, kblk, :],
                            start=(kblk == 0),
                            stop=(kblk == nkblk[j] - 1),
                        )
                    linv = sm_pool.tile([128, 1], f32, tag="linv")
                    nc.vector.reciprocal(linv, po[:, 64:65])
                    nc.vector.tensor_scalar_mul(
                        out_sb[:, g, :], po[:, 0:64], linv
                    )
                nc.sync.dma_start(
                    out=out_c[128 * j : 128 * (j + 1), :, :], in_=out_sb
                )

    nc.compile()
    return nc


def _get_nc(causal: int):
    key = int(causal)
    if key not in _compiled:
        _compiled[key] = _build_nc(key)
    return _compiled[key]


def kernel(xq, xk, xv, W, causal):
    from concourse.bass_utils import run_bass_kernel_spmd

    causal = int(np.asarray(causal))
    nc = _get_nc(causal)

    W = np.asarray(W, dtype=np.float32)
    # block-diagonal mixing weight: wblk[8h+kc, 8g+kc] = W[g,h] / 8
    wblk = np.zeros((128, 128), dtype=np.float32)
    for kc in range(8):
        wblk[kc::8, kc::8] = W.T / 8.0
    ident = np.eye(128, dtype=np.float32)

    in_maps = []
    for cid in range(8):
        b, par = divmod(cid, 2)
        # cm[qc', 8t+kc] = 1 if 8t+kc <= 2qc'+par else 0   (multiplicative)
        qcp = np.arange(128)[:, None]
        kk = np.arange(256)[None, :]
        cm = np.where(kk <= 2 * qcp + par, 1.0, 0.0).astype(np.float32)
        in_maps.append(
            {
                "xq_c": np.ascontiguousarray(xq[b, :, par::2, :], dtype=np.float32),
                "xk_c": np.ascontiguousarray(xk[b], dtype=np.float32),
                "xv_c": np.ascontiguousarray(xv[b], dtype=np.float32),
                "wblk": wblk,
                "cmask": cm,
                "ident": ident,
            }
        )

    global LAST_EXEC_NS, LAST_PROFILE
    res = run_bass_kernel_spmd(nc, in_maps, list(range(8)), trace=TRACE)
    if res.exec_time_ns is not None:
        LAST_EXEC_NS = res.exec_time_ns
        LAST_PROFILE = res.profile_json
    out = np.empty((B, S, H, D), dtype=np.float32)
    for cid in range(8):
        b, par = divmod(cid, 2)
        out[b, par::2, :, :] = res.results[cid]["out_c"]
    return out
